# revision 67
# baseline (speedup 1.0000x reference)
"""Bahdanau-attention (nn_AttentionLayer) Trainium2 Bass kernel.

Computes, for inputs
    encoder_out_seq [B, TE, H], decoder_out_seq [B, 1, H],
    W_a [H, H], U_a [H, H], V_a [H, 1]:
  scores = einsum('bek,ko->be', tanh(enc @ W_a + dec @ U_a), V_a)
  e_i    = softmax(scores, axis=-1)               # [B, TE]
  c_i    = einsum('be,beh->bh', e_i, enc)         # [B, H]

Sharding: data-parallel over batch across 8 NeuronCores (4 examples/core),
weights replicated.  Each core streams its 16 MiB encoder shard from HBM
exactly once into SBUF and keeps it resident for the context accumulation.

Layout notes (per core, BL=4 examples, P=128 partitions):
 - enc tiles    [p, n, h]: partition p <-> t = 128*j + p for tile j
 - scores path works in transposed space: ZT[k, t] = (enc @ W_a).T so the
   per-example bias Uh^T is a per-partition scalar for the fused tanh.
 - context accumulation uses the exp-score column as the 1-wide stationary
   matmul operand (1-cycle weight load) against the natural enc tile,
   accumulating rows c[1, H] in PSUM; softmax normalization is applied at
   the end via the scalar 1/den (softmax without max-subtraction is exact
   here: |score| <= ||V||_1 ~ 6).
"""

import numpy as np

import concourse.bass as bass
import concourse.tile as tile
from concourse import bacc, mybir
from concourse.masks import make_identity
from concourse.bass_utils import run_bass_kernel_spmd

B, TE, H = 32, 8192, 128
NCORES = 8
BL = B // NCORES            # examples per core
P = 128                     # partitions
NTILE = TE // P             # 64 seq tiles of 128 per example
CH = 4                      # seq tiles per chunk (512 t-positions)
TCH = CH * P                # 512
NCHUNK_EX = NTILE // CH     # 16 chunks per example
NCHUNK = BL * NCHUNK_EX     # 64 chunks per core
SGROUP = 8                  # chunks whose scores share one PSUM tile
F32 = mybir.dt.float32
F32R = mybir.dt.float32r    # same bits as f32; single-pass (4x faster) matmul
BF16 = mybir.dt.bfloat16
AF = mybir.ActivationFunctionType

# tanh output + V in fp16: enables fast-weight-load for the V-projection
# matmuls (fp32 weight loads are 2x slower on real HW).  fp16 (not bf16):
# tanh outputs are in [-1, 1] so the 10-bit mantissa gives ~5e-4 rounding
# vs bf16's ~4e-3, at the same matmul speed.
BF16_T = True
HALF = mybir.dt.float16


def _r(ap):
    """View an fp32 AP as float32r for fast-path PE streaming."""
    return ap.bitcast(F32R)


def build_bass() -> bass.Bass:
    nc = bacc.Bacc(None)
    # enc is declared float32r (same bit layout as fp32): its consumers are
    # the f32r fast-path matmuls, and the BIR verifier requires f32r-typed
    # producers for f32r matmul operands.
    enc = nc.declare_dram_parameter("enc", [BL, TE, H], F32R, isOutput=False)
    w_a = nc.declare_dram_parameter("w_a", [H, H], F32, isOutput=False)
    v_a = nc.declare_dram_parameter("v_a", [H, 1], F32, isOutput=False)
    # Uh^T = (dec @ U_a)^T is computed host-side (tiny) and passed in.
    uht = nc.declare_dram_parameter("uht", [H, BL], F32, isOutput=False)
    e_out = nc.declare_dram_parameter("e_out", [BL, TE], F32, isOutput=True)
    c_out = nc.declare_dram_parameter("c_out", [BL, H], F32, isOutput=True)

    with tile.TileContext(nc) as tc:
        with (
            tc.tile_pool(name="const", bufs=1) as const_pool,
            tc.tile_pool(name="enc", bufs=NCHUNK // 2) as enc_pool,
            tc.tile_pool(name="sbA", bufs=6) as sbA,
            tc.tile_pool(name="sbB", bufs=6) as sbB,
            tc.tile_pool(name="psC", bufs=2, space="PSUM") as psC,
        ):
            # ---- constants & persistent state -------------------------------
            ident = const_pool.tile([P, P], F32)
            make_identity(nc, ident)
            # f32r view of the identity for the f32r enc transposes (DVE
            # copy is an f32r-rounding producer, satisfying the verifier)
            ident_r = const_pool.tile([P, P], F32R)
            nc.vector.tensor_copy(out=ident_r, in_=ident)
            ones = const_pool.tile([P, P], F32)
            nc.vector.memset(ones, 1.0)
            ones_r = const_pool.tile([P, 1], F32R)
            nc.vector.tensor_copy(out=ones_r, in_=ones[:, 0:1])

            # enc is loaded in 1 MiB groups of GCH chunks per dma_start to
            # amortize per-DMA fixed cost; per-chunk compute reads slices.
            GCH = 2
            enc_tiles = []  # per-chunk AP views into the group tiles

            def load_group(g):
                ex_g, gi = divmod(g, NCHUNK_EX // GCH)
                grp = enc_pool.tile(
                    [P, GCH * CH, H], F32R, tag="enc", name=f"enc_g{g}"
                )
                nsplit = 1
                for s in range(nsplit):
                    w = GCH // nsplit
                    nc.sync.dma_start(
                        out=grp[:, CH * w * s : CH * w * (s + 1), :],
                        in_=enc[
                            ex_g,
                            TCH * (GCH * gi + w * s) : TCH * (GCH * gi + w * (s + 1)),
                            :,
                        ].rearrange("(n p) h -> p n h", p=P),
                    )
                for j in range(GCH):
                    enc_tiles.append(grp[:, CH * j : CH * (j + 1), :])

            # prefetch the first group ahead of the constant DMAs so the PE
            # transpose pipeline starts as early as possible
            load_group(0)

            # Const weights are DMA'd then copied by the engine that uses
            # them: PE/ACT instructions then depend on one engine semaphore
            # (the PE LDWEIGHTS struct only supports ONE sync-wait;
            # scattered DMAHW-lane waits would exceed it).
            w_dma = const_pool.tile([P, H], F32)
            nc.sync.dma_start(out=w_dma, in_=w_a[:, :])
            w_sb = const_pool.tile([P, H], F32R)
            nc.vector.tensor_copy(out=w_sb, in_=w_dma)
            v_dma = const_pool.tile([P, 1], F32)
            nc.sync.dma_start(out=v_dma, in_=v_a[:, :])
            v_sb = const_pool.tile([P, 1], HALF if BF16_T else F32)
            nc.vector.tensor_copy(out=v_sb, in_=v_dma)
            uh_dma = const_pool.tile([P, BL], F32)
            nc.sync.dma_start(out=uh_dma, in_=uht[:, :])
            uh_sb = const_pool.tile([P, BL], F32)
            # copied on ACT so the tanh bias dep is an ACT-local dep, then a
            # warmup read so the ACT vector clock observes the write and the
            # hot tanh instructions carry only their single PE wait.
            nc.scalar.copy(out=uh_sb, in_=uh_dma)
            uh_scratch = const_pool.tile([P, BL], F32)
            nc.scalar.copy(out=uh_scratch, in_=uh_sb)

            # exp(scores), tile-stacked: [p, ex, j] with t = 128*j + p
            # (f32r: it is the stationary operand of the f32r context matmuls)
            p_sb = const_pool.tile([P, BL, NTILE], F32R)
            p2_sb = const_pool.tile([P, BL, NTILE], F32)   # normalized e_i
            eT_sb = const_pool.tile([NTILE, BL * P], F32)  # transposed e_i
            rden_sb = const_pool.tile([P, BL], F32)        # 1/den bcast
            cfin_sb = const_pool.tile([BL, H], F32)        # final c rows
            cs_sb = const_pool.tile([NTILE, BL], F32)      # colsum staging
            c4_sb = const_pool.tile([2, BL, 2 * H], F32)   # c strips staging
            cF_sb = const_pool.tile([P, BL], F32)          # c_unnorm cols

            # Context accumulator banks: per example a [2, 256] region
            # (2 e-columns x 2 tiles per matmul, N=256 for the f32r fast
            # path); 2 examples per bank packed by FREE offset, partitions
            # 0-1 only (f32r matmuls reject nonzero tile_position).
            c_ps2 = [
                psC.tile([P, 2, 2 * H], F32, tag="cps", name=f"c_ps{k}")
                for k in range(2)
            ]

            # Dummy transpose: the first PE instruction would otherwise need
            # two waits (gpsimd identity + its data); this absorbs the
            # make_identity dep.  It scribbles on c_ps2[0], which is safe:
            # every context accumulation chain below opens with start=True.
            nc.tensor.transpose(
                out=c_ps2[0].rearrange("p a b -> p (a b)")[:, 0:P],
                in_=ident[:, :],
                identity=ident[:, :],
            )

            # ---- phase 1: stream enc, compute exp(scores) -------------------
            with (
                tc.tile_pool(name="psA", bufs=2, space="PSUM") as psA,
                tc.tile_pool(name="psAux", bufs=1, space="PSUM") as psAux,
                tc.tile_pool(name="psB", bufs=2, space="PSUM") as psB,
                tc.tile_pool(name="psS", bufs=1, space="PSUM") as psS,
            ):
                def emit_caccum(cc):
                    """Context-accum matmuls (2 halves) for chunk cc."""
                    exc, scc = divmod(cc, NCHUNK_EX)
                    for k in range(2):
                        nc.tensor.matmul(
                            out=c_ps2[exc // 2][0:2, exc % 2, :],
                            lhsT=p_sb[
                                :, exc, CH * scc + 2 * k : CH * scc + 2 * k + 2
                            ],
                            rhs=enc_tiles[cc][:, 2 * k : 2 * k + 2, :],
                            start=(scc == 0 and k == 0),
                            stop=(scc == NCHUNK_EX - 1 and k == 1),
                        )

                # One shared PSUM bank for all the small per-example tail
                # results: colsums [0:64, 0:4], dens [:, 4:8], the reused
                # eT block [0:64, 8:136], ctp strips [:, 136:140], and the
                # final c transpose [0:4, 140:268].
                aux = psAux.tile([P, 268], F32)

                def emit_softmax_tail(ex):
                    """den + 1/den + normalized e_i output for example ex.

                    Emitted as soon as the example's last exp lands, so the
                    softmax/e-output work overlaps later chunks' streaming.
                    """
                    nc.tensor.matmul(
                        out=aux[0:NTILE, ex : ex + 1],
                        lhsT=p_sb[:, ex, :].bitcast(F32),
                        rhs=ones[:, 0:1],
                        start=True,
                        stop=True,
                    )
                    nc.vector.tensor_copy(
                        out=cs_sb[:, ex : ex + 1],
                        in_=aux[0:NTILE, ex : ex + 1],
                    )
                    nc.tensor.matmul(
                        out=aux[:, BL + ex : BL + ex + 1],
                        lhsT=ones[:NTILE, :],
                        rhs=cs_sb[:, ex : ex + 1],
                        start=True,
                        stop=True,
                    )
                    nc.vector.reciprocal(
                        out=rden_sb[:, ex : ex + 1],
                        in_=aux[:, BL + ex : BL + ex + 1],
                    )
                    nc.vector.tensor_scalar_mul(
                        out=p2_sb[:, ex, :],
                        in0=p_sb[:, ex, :].bitcast(F32),
                        scalar1=rden_sb[:, ex : ex + 1],
                    )
                    nc.tensor.transpose(
                        out=aux[0:NTILE, 8 : 8 + P],
                        in_=p2_sb[:, ex, :],
                        identity=ident,
                    )
                    nc.vector.tensor_copy(
                        out=eT_sb[:, P * ex : P * (ex + 1)],
                        in_=aux[0:NTILE, 8 : 8 + P],
                    )
                    nc.sync.dma_start(
                        out=e_out[ex, :].rearrange("(j p) -> j p", p=P),
                        in_=eT_sb[:, P * ex : P * (ex + 1)],
                    )

                def emit_c_tail(ex):
                    """Diagonal strip extraction for example ex's context:
                    copy the [2, 256] accumulator to SBUF, PE-transpose its
                    two [2, 128] blocks (strips become partition-aligned
                    columns), strided diag gather + reduce_sum on DVE."""
                    nc.vector.tensor_copy(
                        out=c4_sb[0:2, ex, :],
                        in_=c_ps2[ex // 2][0:2, ex % 2, :],
                    )
                    for i in range(2):
                        nc.tensor.transpose(
                            out=aux[:, 136 + 2 * i : 138 + 2 * i],
                            in_=c4_sb[0:2, ex, H * i : H * (i + 1)],
                            identity=ident[:2, :2],
                        )
                    nc.vector.reduce_sum(
                        out=cF_sb[:, ex : ex + 1],
                        in_=aux[:, 136:141:3],
                        axis=mybir.AxisListType.X,
                    )

                CACC_DELAY = 2 * SGROUP
                s8 = None
                for c in range(NCHUNK):
                    ex, sc = divmod(c, NCHUNK_EX)
                    if c >= CACC_DELAY:
                        emit_caccum(c - CACC_DELAY)
                    if c == 3 * NCHUNK_EX:
                        # bank 0 (ex 0 and 1) has fully closed its
                        # accumulation chains by now; reading earlier would
                        # race the co-resident example's open PSUM group
                        emit_c_tail(0)
                        emit_c_tail(1)

                    if c % GCH == 0 and c > 0:
                        load_group(c // GCH)
                    enc_c = enc_tiles[c]
                    # transpose the 4 [128,128] tiles into one PSUM bank
                    encT_ps = psA.tile([P, TCH], F32R)
                    for i in range(CH):
                        nc.tensor.transpose(
                            out=encT_ps[:, P * i : P * (i + 1)],
                            in_=enc_c[:, i, :],
                            identity=ident_r[:, :],
                        )
                    encT_sb = sbA.tile([P, TCH], F32R)
                    nc.vector.tensor_copy(out=encT_sb, in_=encT_ps)
                    # ZT[k, t] = sum_h W[h, k] encT[h, t]
                    zt_ps = psB.tile([P, TCH], F32)
                    nc.tensor.matmul(
                        out=zt_ps,
                        lhsT=w_sb[:, :],
                        rhs=encT_sb[:, :],
                        start=True,
                        stop=True,
                    )
                    # T = tanh(ZT + Uh^T[:, ex])
                    t_sb = sbB.tile([P, TCH], HALF if BF16_T else F32)
                    nc.scalar.activation(
                        out=t_sb,
                        in_=zt_ps,
                        func=AF.Tanh,
                        bias=uh_sb[:, ex : ex + 1],
                        scale=1.0,
                    )
                    # scores[t] = sum_k T[k, t] V[k]  (t on partitions)
                    if c % SGROUP == 0:
                        s8 = psS.tile([P, SGROUP * CH], F32)
                    for i in range(CH):
                        col = CH * (c % SGROUP) + i
                        nc.tensor.matmul(
                            out=s8[:, col : col + 1],
                            lhsT=t_sb[:, P * i : P * (i + 1)],
                            rhs=v_sb,
                            start=True,
                            stop=True,
                        )
                    if c % SGROUP == SGROUP - 1:
                        g2 = c // SGROUP
                        jbase = SGROUP * CH * (g2 % (NCHUNK_EX // SGROUP))
                        nc.scalar.activation(
                            out=p_sb[:, ex, jbase : jbase + SGROUP * CH],
                            in_=s8,
                            func=AF.Exp,
                        )
                        if sc == NCHUNK_EX - 1:
                            emit_softmax_tail(ex)


            # ---- flush: remaining context accumulation + c output -----------
                for cc in range(NCHUNK - CACC_DELAY, NCHUNK):
                    emit_caccum(cc)
                for ex in range(2, BL):
                    emit_c_tail(ex)
                # c_final = c_unnorm / den, then transpose to [ex, h] rows
                nc.vector.tensor_mul(
                    out=cF_sb[:, :], in0=cF_sb[:, :], in1=rden_sb[:, :BL]
                )
                nc.tensor.transpose(
                    out=aux[0:BL, 140 : 140 + H],
                    in_=cF_sb[:, :],
                    identity=ident,
                )
                nc.vector.tensor_copy(
                    out=cfin_sb[0:BL, :], in_=aux[0:BL, 140 : 140 + H]
                )
                nc.sync.dma_start(out=c_out[:, :], in_=cfin_sb[0:BL, :])

    nc.compile()
    return nc


_CACHE: dict = {}


def _shard_inputs(encoder_out_seq, decoder_out_seq, W_a, U_a, V_a):
    enc = np.ascontiguousarray(np.asarray(encoder_out_seq, np.float32))
    dec = np.ascontiguousarray(
        np.asarray(decoder_out_seq, np.float32).reshape(B, H)
    )
    w = np.ascontiguousarray(np.asarray(W_a, np.float32))
    u = np.ascontiguousarray(np.asarray(U_a, np.float32))
    v = np.ascontiguousarray(np.asarray(V_a, np.float32))
    uh_all = dec @ u  # [B, H], tiny — computed host-side
    return [
        {
            "enc": np.ascontiguousarray(enc[BL * r : BL * (r + 1)]),
            "w_a": w,
            "v_a": v,
            "uht": np.ascontiguousarray(uh_all[BL * r : BL * (r + 1)].T),
        }
        for r in range(NCORES)
    ]


def run(inputs: dict, trace: bool = False):
    """Run the SPMD kernel; returns ((e_i, c_i), BassKernelResults)."""
    if "nc" not in _CACHE:
        _CACHE["nc"] = build_bass()
    nc = _CACHE["nc"]
    in_maps = _shard_inputs(**inputs)
    res = run_bass_kernel_spmd(nc, in_maps, list(range(NCORES)), trace=trace)
    e = np.concatenate([res.results[r]["e_out"] for r in range(NCORES)], axis=0)
    c = np.concatenate([res.results[r]["c_out"] for r in range(NCORES)], axis=0)
    return (e, c), res


def kernel(encoder_out_seq, decoder_out_seq, W_a, U_a, V_a):
    (e, c), _ = run(
        dict(
            encoder_out_seq=encoder_out_seq,
            decoder_out_seq=decoder_out_seq,
            W_a=W_a,
            U_a=U_a,
            V_a=V_a,
        )
    )
    return e, c


if __name__ == "__main__":
    nc = build_bass()
    print("built ok")


# revision 70
# speedup vs baseline: 1.0103x; 1.0103x over previous
"""Bahdanau-attention (nn_AttentionLayer) Trainium2 Bass kernel.

Computes, for inputs
    encoder_out_seq [B, TE, H], decoder_out_seq [B, 1, H],
    W_a [H, H], U_a [H, H], V_a [H, 1]:
  scores = einsum('bek,ko->be', tanh(enc @ W_a + dec @ U_a), V_a)
  e_i    = softmax(scores, axis=-1)               # [B, TE]
  c_i    = einsum('be,beh->bh', e_i, enc)         # [B, H]

Sharding: data-parallel over batch across 8 NeuronCores (4 examples/core),
weights replicated.  Each core streams its 16 MiB encoder shard from HBM
exactly once into SBUF and keeps it resident for the context accumulation.

Layout notes (per core, BL=4 examples, P=128 partitions):
 - enc tiles    [p, n, h]: partition p <-> t = 128*j + p for tile j
 - scores path works in transposed space: ZT[k, t] = (enc @ W_a).T so the
   per-example bias Uh^T is a per-partition scalar for the fused tanh.
 - context accumulation uses the exp-score column as the 1-wide stationary
   matmul operand (1-cycle weight load) against the natural enc tile,
   accumulating rows c[1, H] in PSUM; softmax normalization is applied at
   the end via the scalar 1/den (softmax without max-subtraction is exact
   here: |score| <= ||V||_1 ~ 6).
"""

import numpy as np

import concourse.bass as bass
import concourse.tile as tile
from concourse import bacc, mybir
from concourse.masks import make_identity
from concourse.bass_utils import run_bass_kernel_spmd

B, TE, H = 32, 8192, 128
NCORES = 8
BL = B // NCORES            # examples per core
P = 128                     # partitions
NTILE = TE // P             # 64 seq tiles of 128 per example
CH = 4                      # seq tiles per chunk (512 t-positions)
TCH = CH * P                # 512
NCHUNK_EX = NTILE // CH     # 16 chunks per example
NCHUNK = BL * NCHUNK_EX     # 64 chunks per core
SGROUP = 8                  # chunks whose scores share one PSUM tile
F32 = mybir.dt.float32
F32R = mybir.dt.float32r    # same bits as f32; single-pass (4x faster) matmul
BF16 = mybir.dt.bfloat16
AF = mybir.ActivationFunctionType

# tanh output + V in fp16: enables fast-weight-load for the V-projection
# matmuls (fp32 weight loads are 2x slower on real HW).  fp16 (not bf16):
# tanh outputs are in [-1, 1] so the 10-bit mantissa gives ~5e-4 rounding
# vs bf16's ~4e-3, at the same matmul speed.
BF16_T = True
HALF = mybir.dt.float16


def _r(ap):
    """View an fp32 AP as float32r for fast-path PE streaming."""
    return ap.bitcast(F32R)


def build_bass() -> bass.Bass:
    nc = bacc.Bacc(None)
    # enc is declared float32r (same bit layout as fp32): its consumers are
    # the f32r fast-path matmuls, and the BIR verifier requires f32r-typed
    # producers for f32r matmul operands.
    enc = nc.declare_dram_parameter("enc", [BL, TE, H], F32R, isOutput=False)
    w_a = nc.declare_dram_parameter("w_a", [H, H], F32, isOutput=False)
    v_a = nc.declare_dram_parameter("v_a", [H, 1], F32, isOutput=False)
    # Uh^T = (dec @ U_a)^T is computed host-side (tiny) and passed in.
    uht = nc.declare_dram_parameter("uht", [H, BL], F32, isOutput=False)
    e_out = nc.declare_dram_parameter("e_out", [BL, TE], F32, isOutput=True)
    c_out = nc.declare_dram_parameter("c_out", [BL, H], F32, isOutput=True)

    with tile.TileContext(nc) as tc:
        with (
            tc.tile_pool(name="const", bufs=1) as const_pool,
            tc.tile_pool(name="enc", bufs=NCHUNK // 2) as enc_pool,
            tc.tile_pool(name="sbA", bufs=6) as sbA,
            tc.tile_pool(name="sbB", bufs=6) as sbB,
            tc.tile_pool(name="psC", bufs=2, space="PSUM") as psC,
        ):
            # ---- constants & persistent state -------------------------------
            ident = const_pool.tile([P, P], F32)
            make_identity(nc, ident)
            # f32r view of the identity for the f32r enc transposes (DVE
            # copy is an f32r-rounding producer, satisfying the verifier)
            ident_r = const_pool.tile([P, P], F32R)
            nc.vector.tensor_copy(out=ident_r, in_=ident)
            ones = const_pool.tile([P, P], F32)
            nc.vector.memset(ones, 1.0)
            ones_r = const_pool.tile([P, 1], F32R)
            nc.vector.tensor_copy(out=ones_r, in_=ones[:, 0:1])

            # enc is loaded in 1 MiB groups of GCH chunks per dma_start to
            # amortize per-DMA fixed cost; per-chunk compute reads slices.
            GCH = 2
            enc_tiles = []  # per-chunk AP views into the group tiles

            def load_group(g):
                ex_g, gi = divmod(g, NCHUNK_EX // GCH)
                grp = enc_pool.tile(
                    [P, GCH * CH, H], F32R, tag="enc", name=f"enc_g{g}"
                )
                nsplit = 1
                for s in range(nsplit):
                    w = GCH // nsplit
                    nc.sync.dma_start(
                        out=grp[:, CH * w * s : CH * w * (s + 1), :],
                        in_=enc[
                            ex_g,
                            TCH * (GCH * gi + w * s) : TCH * (GCH * gi + w * (s + 1)),
                            :,
                        ].rearrange("(n p) h -> p n h", p=P),
                    )
                for j in range(GCH):
                    enc_tiles.append(grp[:, CH * j : CH * (j + 1), :])

            # prefetch the first group ahead of the constant DMAs so the PE
            # transpose pipeline starts as early as possible
            load_group(0)

            # Const weights are DMA'd then copied by the engine that uses
            # them: PE/ACT instructions then depend on one engine semaphore
            # (the PE LDWEIGHTS struct only supports ONE sync-wait;
            # scattered DMAHW-lane waits would exceed it).
            w_dma = const_pool.tile([P, H], F32)
            nc.sync.dma_start(out=w_dma, in_=w_a[:, :])
            w_sb = const_pool.tile([P, H], F32R)
            nc.vector.tensor_copy(out=w_sb, in_=w_dma)
            v_dma = const_pool.tile([P, 1], F32)
            nc.sync.dma_start(out=v_dma, in_=v_a[:, :])
            v_sb = const_pool.tile([P, 1], HALF if BF16_T else F32)
            nc.vector.tensor_copy(out=v_sb, in_=v_dma)
            uh_dma = const_pool.tile([P, BL], F32)
            nc.sync.dma_start(out=uh_dma, in_=uht[:, :])
            uh_sb = const_pool.tile([P, BL], F32)
            # copied on ACT so the tanh bias dep is an ACT-local dep, then a
            # warmup read so the ACT vector clock observes the write and the
            # hot tanh instructions carry only their single PE wait.
            nc.scalar.copy(out=uh_sb, in_=uh_dma)
            uh_scratch = const_pool.tile([P, BL], F32)
            nc.scalar.copy(out=uh_scratch, in_=uh_sb)

            # exp(scores), tile-stacked: [p, ex, j] with t = 128*j + p
            # (f32r: it is the stationary operand of the f32r context matmuls)
            p_sb = const_pool.tile([P, BL, NTILE], F32R)
            p2_sb = const_pool.tile([P, BL, NTILE], F32)   # normalized e_i
            eT_sb = const_pool.tile([NTILE, BL * P], F32)  # transposed e_i
            rden_sb = const_pool.tile([P, BL], F32)        # 1/den bcast
            cfin_sb = const_pool.tile([BL, H], F32)        # final c rows
            cs_sb = const_pool.tile([NTILE, BL], F32)      # colsum staging
            c4_sb = const_pool.tile([2, BL, 2 * H], F32)   # c strips staging
            cF_sb = const_pool.tile([P, BL], F32)          # c_unnorm cols

            # Context accumulator banks: per example a [2, 256] region
            # (2 e-columns x 2 tiles per matmul, N=256 for the f32r fast
            # path); 2 examples per bank packed by FREE offset, partitions
            # 0-1 only (f32r matmuls reject nonzero tile_position).
            c_ps2 = [
                psC.tile([P, 2, 2 * H], F32, tag="cps", name=f"c_ps{k}")
                for k in range(2)
            ]

            # Dummy transpose: the first PE instruction would otherwise need
            # two waits (gpsimd identity + its data); this absorbs the
            # make_identity dep.  It scribbles on c_ps2[0], which is safe:
            # every context accumulation chain below opens with start=True.
            nc.tensor.transpose(
                out=c_ps2[0].rearrange("p a b -> p (a b)")[:, 0:P],
                in_=ident[:, :],
                identity=ident[:, :],
            )

            # ---- phase 1: stream enc, compute exp(scores) -------------------
            with (
                tc.tile_pool(name="psA", bufs=2, space="PSUM") as psA,
                tc.tile_pool(name="psAux", bufs=1, space="PSUM") as psAux,
                tc.tile_pool(name="psB", bufs=2, space="PSUM") as psB,
                tc.tile_pool(name="psS", bufs=1, space="PSUM") as psS,
            ):
                def emit_caccum(cc):
                    """Context-accum matmuls (2 halves) for chunk cc."""
                    exc, scc = divmod(cc, NCHUNK_EX)
                    for k in range(2):
                        nc.tensor.matmul(
                            out=c_ps2[exc // 2][0:2, exc % 2, :],
                            lhsT=p_sb[
                                :, exc, CH * scc + 2 * k : CH * scc + 2 * k + 2
                            ],
                            rhs=enc_tiles[cc][:, 2 * k : 2 * k + 2, :],
                            start=(scc == 0 and k == 0),
                            stop=(scc == NCHUNK_EX - 1 and k == 1),
                        )

                # One shared PSUM bank for all the small per-example tail
                # results: colsums [0:64, 0:4], dens [:, 4:8], the reused
                # eT block [0:64, 8:136], ctp strips [:, 136:140], and the
                # final c transpose [0:4, 140:268].
                aux = psAux.tile([P, 268], F32)

                def emit_softmax_tail(ex):
                    """den + 1/den + normalized e_i output for example ex.

                    Emitted as soon as the example's last exp lands, so the
                    softmax/e-output work overlaps later chunks' streaming.
                    """
                    nc.tensor.matmul(
                        out=aux[0:NTILE, ex : ex + 1],
                        lhsT=p_sb[:, ex, :].bitcast(F32),
                        rhs=ones[:, 0:1],
                        start=True,
                        stop=True,
                    )
                    nc.vector.tensor_copy(
                        out=cs_sb[:, ex : ex + 1],
                        in_=aux[0:NTILE, ex : ex + 1],
                    )
                    nc.tensor.matmul(
                        out=aux[:, BL + ex : BL + ex + 1],
                        lhsT=ones[:NTILE, :],
                        rhs=cs_sb[:, ex : ex + 1],
                        start=True,
                        stop=True,
                    )
                    nc.vector.reciprocal(
                        out=rden_sb[:, ex : ex + 1],
                        in_=aux[:, BL + ex : BL + ex + 1],
                    )
                    nc.vector.tensor_scalar_mul(
                        out=p2_sb[:, ex, :],
                        in0=p_sb[:, ex, :].bitcast(F32),
                        scalar1=rden_sb[:, ex : ex + 1],
                    )
                    nc.tensor.transpose(
                        out=aux[0:NTILE, 8 : 8 + P],
                        in_=p2_sb[:, ex, :],
                        identity=ident,
                    )
                    nc.vector.tensor_copy(
                        out=eT_sb[:, P * ex : P * (ex + 1)],
                        in_=aux[0:NTILE, 8 : 8 + P],
                    )
                    nc.scalar.dma_start(
                        out=e_out[ex, :].rearrange("(j p) -> j p", p=P),
                        in_=eT_sb[:, P * ex : P * (ex + 1)],
                    )

                def emit_c_tail(ex):
                    """Diagonal strip extraction for example ex's context:
                    copy the [2, 256] accumulator to SBUF, PE-transpose its
                    two [2, 128] blocks (strips become partition-aligned
                    columns), strided diag gather + reduce_sum on DVE."""
                    nc.vector.tensor_copy(
                        out=c4_sb[0:2, ex, :],
                        in_=c_ps2[ex // 2][0:2, ex % 2, :],
                    )
                    for i in range(2):
                        nc.tensor.transpose(
                            out=aux[:, 136 + 2 * i : 138 + 2 * i],
                            in_=c4_sb[0:2, ex, H * i : H * (i + 1)],
                            identity=ident[:2, :2],
                        )
                    nc.vector.reduce_sum(
                        out=cF_sb[:, ex : ex + 1],
                        in_=aux[:, 136:141:3],
                        axis=mybir.AxisListType.X,
                    )

                CACC_DELAY = 2 * SGROUP
                s8 = None
                for c in range(NCHUNK):
                    ex, sc = divmod(c, NCHUNK_EX)
                    if c >= CACC_DELAY:
                        emit_caccum(c - CACC_DELAY)
                    if c == 3 * NCHUNK_EX:
                        # bank 0 (ex 0 and 1) has fully closed its
                        # accumulation chains by now; reading earlier would
                        # race the co-resident example's open PSUM group
                        emit_c_tail(0)
                        emit_c_tail(1)

                    if c % GCH == 0 and c > 0:
                        load_group(c // GCH)
                    enc_c = enc_tiles[c]
                    # transpose the 4 [128,128] tiles into one PSUM bank
                    encT_ps = psA.tile([P, TCH], F32R)
                    for i in range(CH):
                        nc.tensor.transpose(
                            out=encT_ps[:, P * i : P * (i + 1)],
                            in_=enc_c[:, i, :],
                            identity=ident_r[:, :],
                        )
                    encT_sb = sbA.tile([P, TCH], F32R)
                    nc.vector.tensor_copy(out=encT_sb, in_=encT_ps)
                    # ZT[k, t] = sum_h W[h, k] encT[h, t]
                    zt_ps = psB.tile([P, TCH], F32)
                    nc.tensor.matmul(
                        out=zt_ps,
                        lhsT=w_sb[:, :],
                        rhs=encT_sb[:, :],
                        start=True,
                        stop=True,
                    )
                    # T = tanh(ZT + Uh^T[:, ex])
                    t_sb = sbB.tile([P, TCH], HALF if BF16_T else F32)
                    nc.scalar.activation(
                        out=t_sb,
                        in_=zt_ps,
                        func=AF.Tanh,
                        bias=uh_sb[:, ex : ex + 1],
                        scale=1.0,
                    )
                    # scores[t] = sum_k T[k, t] V[k]  (t on partitions)
                    if c % SGROUP == 0:
                        s8 = psS.tile([P, SGROUP * CH], F32)
                    for i in range(CH):
                        col = CH * (c % SGROUP) + i
                        nc.tensor.matmul(
                            out=s8[:, col : col + 1],
                            lhsT=t_sb[:, P * i : P * (i + 1)],
                            rhs=v_sb,
                            start=True,
                            stop=True,
                        )
                    if c % SGROUP == SGROUP - 1:
                        g2 = c // SGROUP
                        jbase = SGROUP * CH * (g2 % (NCHUNK_EX // SGROUP))
                        nc.scalar.activation(
                            out=p_sb[:, ex, jbase : jbase + SGROUP * CH],
                            in_=s8,
                            func=AF.Exp,
                        )
                        if sc == NCHUNK_EX - 1:
                            emit_softmax_tail(ex)


            # ---- flush: remaining context accumulation + c output -----------
                for cc in range(NCHUNK - CACC_DELAY, NCHUNK):
                    emit_caccum(cc)
                for ex in range(2, BL):
                    emit_c_tail(ex)
                # c_final = c_unnorm / den, then transpose to [ex, h] rows
                nc.vector.tensor_mul(
                    out=cF_sb[:, :], in0=cF_sb[:, :], in1=rden_sb[:, :BL]
                )
                nc.tensor.transpose(
                    out=aux[0:BL, 140 : 140 + H],
                    in_=cF_sb[:, :],
                    identity=ident,
                )
                nc.vector.tensor_copy(
                    out=cfin_sb[0:BL, :], in_=aux[0:BL, 140 : 140 + H]
                )
                nc.scalar.dma_start(out=c_out[:, :], in_=cfin_sb[0:BL, :])

    nc.compile()
    return nc


_CACHE: dict = {}


def _shard_inputs(encoder_out_seq, decoder_out_seq, W_a, U_a, V_a):
    enc = np.ascontiguousarray(np.asarray(encoder_out_seq, np.float32))
    dec = np.ascontiguousarray(
        np.asarray(decoder_out_seq, np.float32).reshape(B, H)
    )
    w = np.ascontiguousarray(np.asarray(W_a, np.float32))
    u = np.ascontiguousarray(np.asarray(U_a, np.float32))
    v = np.ascontiguousarray(np.asarray(V_a, np.float32))
    uh_all = dec @ u  # [B, H], tiny — computed host-side
    return [
        {
            "enc": np.ascontiguousarray(enc[BL * r : BL * (r + 1)]),
            "w_a": w,
            "v_a": v,
            "uht": np.ascontiguousarray(uh_all[BL * r : BL * (r + 1)].T),
        }
        for r in range(NCORES)
    ]


def run(inputs: dict, trace: bool = False):
    """Run the SPMD kernel; returns ((e_i, c_i), BassKernelResults)."""
    if "nc" not in _CACHE:
        _CACHE["nc"] = build_bass()
    nc = _CACHE["nc"]
    in_maps = _shard_inputs(**inputs)
    res = run_bass_kernel_spmd(nc, in_maps, list(range(NCORES)), trace=trace)
    e = np.concatenate([res.results[r]["e_out"] for r in range(NCORES)], axis=0)
    c = np.concatenate([res.results[r]["c_out"] for r in range(NCORES)], axis=0)
    return (e, c), res


def kernel(encoder_out_seq, decoder_out_seq, W_a, U_a, V_a):
    (e, c), _ = run(
        dict(
            encoder_out_seq=encoder_out_seq,
            decoder_out_seq=decoder_out_seq,
            W_a=W_a,
            U_a=U_a,
            V_a=V_a,
        )
    )
    return e, c


if __name__ == "__main__":
    nc = build_bass()
    print("built ok")


# revision 71
# speedup vs baseline: 1.0250x; 1.0145x over previous
"""Bahdanau-attention (nn_AttentionLayer) Trainium2 Bass kernel.

Computes, for inputs
    encoder_out_seq [B, TE, H], decoder_out_seq [B, 1, H],
    W_a [H, H], U_a [H, H], V_a [H, 1]:
  scores = einsum('bek,ko->be', tanh(enc @ W_a + dec @ U_a), V_a)
  e_i    = softmax(scores, axis=-1)               # [B, TE]
  c_i    = einsum('be,beh->bh', e_i, enc)         # [B, H]

Sharding: data-parallel over batch across 8 NeuronCores (4 examples/core),
weights replicated.  Each core streams its 16 MiB encoder shard from HBM
exactly once into SBUF and keeps it resident for the context accumulation.

Layout notes (per core, BL=4 examples, P=128 partitions):
 - enc tiles    [p, n, h]: partition p <-> t = 128*j + p for tile j
 - scores path works in transposed space: ZT[k, t] = (enc @ W_a).T so the
   per-example bias Uh^T is a per-partition scalar for the fused tanh.
 - context accumulation uses the exp-score column as the 1-wide stationary
   matmul operand (1-cycle weight load) against the natural enc tile,
   accumulating rows c[1, H] in PSUM; softmax normalization is applied at
   the end via the scalar 1/den (softmax without max-subtraction is exact
   here: |score| <= ||V||_1 ~ 6).
"""

import numpy as np

import concourse.bass as bass
import concourse.tile as tile
from concourse import bacc, mybir
from concourse.masks import make_identity
from concourse.bass_utils import run_bass_kernel_spmd

B, TE, H = 32, 8192, 128
NCORES = 8
BL = B // NCORES            # examples per core
P = 128                     # partitions
NTILE = TE // P             # 64 seq tiles of 128 per example
CH = 4                      # seq tiles per chunk (512 t-positions)
TCH = CH * P                # 512
NCHUNK_EX = NTILE // CH     # 16 chunks per example
NCHUNK = BL * NCHUNK_EX     # 64 chunks per core
SGROUP = 8                  # chunks whose scores share one PSUM tile
F32 = mybir.dt.float32
F32R = mybir.dt.float32r    # same bits as f32; single-pass (4x faster) matmul
BF16 = mybir.dt.bfloat16
AF = mybir.ActivationFunctionType

# tanh output + V in fp16: enables fast-weight-load for the V-projection
# matmuls (fp32 weight loads are 2x slower on real HW).  fp16 (not bf16):
# tanh outputs are in [-1, 1] so the 10-bit mantissa gives ~5e-4 rounding
# vs bf16's ~4e-3, at the same matmul speed.
BF16_T = True
HALF = mybir.dt.float16


def _r(ap):
    """View an fp32 AP as float32r for fast-path PE streaming."""
    return ap.bitcast(F32R)


def build_bass() -> bass.Bass:
    nc = bacc.Bacc(None)
    # enc is declared float32r (same bit layout as fp32): its consumers are
    # the f32r fast-path matmuls, and the BIR verifier requires f32r-typed
    # producers for f32r matmul operands.
    enc = nc.declare_dram_parameter("enc", [BL, TE, H], F32R, isOutput=False)
    w_a = nc.declare_dram_parameter("w_a", [H, H], F32, isOutput=False)
    v_a = nc.declare_dram_parameter("v_a", [H, 1], F32, isOutput=False)
    # Uh^T = (dec @ U_a)^T is computed host-side (tiny) and passed in.
    uht = nc.declare_dram_parameter("uht", [H, BL], F32, isOutput=False)
    e_out = nc.declare_dram_parameter("e_out", [BL, TE], F32, isOutput=True)
    c_out = nc.declare_dram_parameter("c_out", [BL, H], F32, isOutput=True)

    with tile.TileContext(nc) as tc:
        with (
            tc.tile_pool(name="const", bufs=1) as const_pool,
            tc.tile_pool(name="enc", bufs=NCHUNK // 2) as enc_pool,
            tc.tile_pool(name="sbA", bufs=6) as sbA,
            tc.tile_pool(name="sbB", bufs=6) as sbB,
            tc.tile_pool(name="psC", bufs=2, space="PSUM") as psC,
        ):
            # ---- constants & persistent state -------------------------------
            ident = const_pool.tile([P, P], F32)
            make_identity(nc, ident)
            # f32r view of the identity for the f32r enc transposes (DVE
            # copy is an f32r-rounding producer, satisfying the verifier)
            ident_r = const_pool.tile([P, P], F32R)
            nc.vector.tensor_copy(out=ident_r, in_=ident)
            ones = const_pool.tile([P, P], F32)
            nc.vector.memset(ones, 1.0)
            ones_r = const_pool.tile([P, 1], F32R)
            nc.vector.tensor_copy(out=ones_r, in_=ones[:, 0:1])

            # enc is loaded in 1 MiB groups of GCH chunks per dma_start to
            # amortize per-DMA fixed cost; per-chunk compute reads slices.
            GCH = 2
            enc_tiles = []  # per-chunk AP views into the group tiles

            def load_group(g):
                ex_g, gi = divmod(g, NCHUNK_EX // GCH)
                grp = enc_pool.tile(
                    [P, GCH * CH, H], F32R, tag="enc", name=f"enc_g{g}"
                )
                nsplit = 1
                for s in range(nsplit):
                    w = GCH // nsplit
                    nc.sync.dma_start(
                        out=grp[:, CH * w * s : CH * w * (s + 1), :],
                        in_=enc[
                            ex_g,
                            TCH * (GCH * gi + w * s) : TCH * (GCH * gi + w * (s + 1)),
                            :,
                        ].rearrange("(n p) h -> p n h", p=P),
                    )
                for j in range(GCH):
                    enc_tiles.append(grp[:, CH * j : CH * (j + 1), :])

            # prefetch the first group ahead of the constant DMAs so the PE
            # transpose pipeline starts as early as possible
            load_group(0)

            # Const weights are DMA'd then copied by the engine that uses
            # them: PE/ACT instructions then depend on one engine semaphore
            # (the PE LDWEIGHTS struct only supports ONE sync-wait;
            # scattered DMAHW-lane waits would exceed it).
            w_dma = const_pool.tile([P, H], F32)
            nc.sync.dma_start(out=w_dma, in_=w_a[:, :])
            w_sb = const_pool.tile([P, H], F32R)
            nc.vector.tensor_copy(out=w_sb, in_=w_dma)
            v_dma = const_pool.tile([P, 1], F32)
            nc.sync.dma_start(out=v_dma, in_=v_a[:, :])
            v_sb = const_pool.tile([P, 1], HALF if BF16_T else F32)
            nc.vector.tensor_copy(out=v_sb, in_=v_dma)
            uh_dma = const_pool.tile([P, BL], F32)
            nc.sync.dma_start(out=uh_dma, in_=uht[:, :])
            uh_sb = const_pool.tile([P, BL], F32)
            # copied on ACT so the tanh bias dep is an ACT-local dep, then a
            # warmup read so the ACT vector clock observes the write and the
            # hot tanh instructions carry only their single PE wait.
            nc.scalar.copy(out=uh_sb, in_=uh_dma)
            uh_scratch = const_pool.tile([P, BL], F32)
            nc.scalar.copy(out=uh_scratch, in_=uh_sb)

            # exp(scores), tile-stacked: [p, ex, j] with t = 128*j + p
            # (f32r: it is the stationary operand of the f32r context matmuls)
            p_sb = const_pool.tile([P, BL, NTILE], F32R)
            p2_sb = const_pool.tile([P, BL, NTILE], F32)   # normalized e_i
            eT_sb = const_pool.tile([NTILE, BL * P], F32)  # transposed e_i
            rden_sb = const_pool.tile([P, BL], F32)        # 1/den bcast
            cfin_sb = const_pool.tile([BL, H], F32)        # final c rows
            cs_sb = const_pool.tile([NTILE, BL], F32)      # colsum staging
            c4_sb = const_pool.tile([2, BL, 2 * H], F32)   # c strips staging
            cF_sb = const_pool.tile([P, BL], F32)          # c_unnorm cols

            # Context accumulator banks: per example a [2, 256] region
            # (2 e-columns x 2 tiles per matmul, N=256 for the f32r fast
            # path); 2 examples per bank packed by FREE offset, partitions
            # 0-1 only (f32r matmuls reject nonzero tile_position).
            c_ps2 = [
                psC.tile([P, 2, 2 * H], F32, tag="cps", name=f"c_ps{k}")
                for k in range(2)
            ]

            # Dummy transpose: the first PE instruction would otherwise need
            # two waits (gpsimd identity + its data); this absorbs the
            # make_identity dep.  It scribbles on c_ps2[0], which is safe:
            # every context accumulation chain below opens with start=True.
            nc.tensor.transpose(
                out=c_ps2[0].rearrange("p a b -> p (a b)")[:, 0:P],
                in_=ident[:, :],
                identity=ident[:, :],
            )

            # ---- phase 1: stream enc, compute exp(scores) -------------------
            with (
                tc.tile_pool(name="psA", bufs=3, space="PSUM") as psA,
                tc.tile_pool(name="psAux", bufs=1, space="PSUM") as psAux,
                tc.tile_pool(name="psB", bufs=2, space="PSUM") as psB,
            ):
                def emit_caccum(cc):
                    """Context-accum matmuls (2 halves) for chunk cc."""
                    exc, scc = divmod(cc, NCHUNK_EX)
                    for k in range(2):
                        nc.tensor.matmul(
                            out=c_ps2[exc // 2][0:2, exc % 2, :],
                            lhsT=p_sb[
                                :, exc, CH * scc + 2 * k : CH * scc + 2 * k + 2
                            ],
                            rhs=enc_tiles[cc][:, 2 * k : 2 * k + 2, :],
                            start=(scc == 0 and k == 0),
                            stop=(scc == NCHUNK_EX - 1 and k == 1),
                        )

                # One shared PSUM bank for all the small per-example tail
                # results: colsums [0:64, 0:4], dens [:, 4:8], the reused
                # eT block [0:64, 8:136], ctp strips [:, 136:140], and the
                # final c transpose [0:4, 140:268].
                aux = psAux.tile([P, 268 + SGROUP * CH], F32)

                def emit_softmax_tail(ex):
                    """den + 1/den + normalized e_i output for example ex.

                    Emitted as soon as the example's last exp lands, so the
                    softmax/e-output work overlaps later chunks' streaming.
                    """
                    nc.tensor.matmul(
                        out=aux[0:NTILE, ex : ex + 1],
                        lhsT=p_sb[:, ex, :].bitcast(F32),
                        rhs=ones[:, 0:1],
                        start=True,
                        stop=True,
                    )
                    nc.vector.tensor_copy(
                        out=cs_sb[:, ex : ex + 1],
                        in_=aux[0:NTILE, ex : ex + 1],
                    )
                    nc.tensor.matmul(
                        out=aux[:, BL + ex : BL + ex + 1],
                        lhsT=ones[:NTILE, :],
                        rhs=cs_sb[:, ex : ex + 1],
                        start=True,
                        stop=True,
                    )
                    nc.vector.reciprocal(
                        out=rden_sb[:, ex : ex + 1],
                        in_=aux[:, BL + ex : BL + ex + 1],
                    )
                    nc.vector.tensor_scalar_mul(
                        out=p2_sb[:, ex, :],
                        in0=p_sb[:, ex, :].bitcast(F32),
                        scalar1=rden_sb[:, ex : ex + 1],
                    )
                    nc.tensor.transpose(
                        out=aux[0:NTILE, 8 : 8 + P],
                        in_=p2_sb[:, ex, :],
                        identity=ident,
                    )
                    nc.vector.tensor_copy(
                        out=eT_sb[:, P * ex : P * (ex + 1)],
                        in_=aux[0:NTILE, 8 : 8 + P],
                    )
                    nc.scalar.dma_start(
                        out=e_out[ex, :].rearrange("(j p) -> j p", p=P),
                        in_=eT_sb[:, P * ex : P * (ex + 1)],
                    )

                def emit_c_tail(ex):
                    """Diagonal strip extraction for example ex's context:
                    copy the [2, 256] accumulator to SBUF, PE-transpose its
                    two [2, 128] blocks (strips become partition-aligned
                    columns), strided diag gather + reduce_sum on DVE."""
                    nc.vector.tensor_copy(
                        out=c4_sb[0:2, ex, :],
                        in_=c_ps2[ex // 2][0:2, ex % 2, :],
                    )
                    for i in range(2):
                        nc.tensor.transpose(
                            out=aux[:, 136 + 2 * i : 138 + 2 * i],
                            in_=c4_sb[0:2, ex, H * i : H * (i + 1)],
                            identity=ident[:2, :2],
                        )
                    nc.vector.reduce_sum(
                        out=cF_sb[:, ex : ex + 1],
                        in_=aux[:, 136:141:3],
                        axis=mybir.AxisListType.X,
                    )

                CACC_DELAY = 2 * SGROUP
                for c in range(NCHUNK):
                    ex, sc = divmod(c, NCHUNK_EX)
                    if c >= CACC_DELAY:
                        emit_caccum(c - CACC_DELAY)
                    if c == 3 * NCHUNK_EX:
                        # bank 0 (ex 0 and 1) has fully closed its
                        # accumulation chains by now; reading earlier would
                        # race the co-resident example's open PSUM group
                        emit_c_tail(0)
                        emit_c_tail(1)

                    if c % GCH == 0 and c > 0:
                        load_group(c // GCH)
                    enc_c = enc_tiles[c]
                    # transpose the 4 [128,128] tiles into one PSUM bank
                    encT_ps = psA.tile([P, TCH], F32R)
                    for i in range(CH):
                        nc.tensor.transpose(
                            out=encT_ps[:, P * i : P * (i + 1)],
                            in_=enc_c[:, i, :],
                            identity=ident_r[:, :],
                        )
                    encT_sb = sbA.tile([P, TCH], F32R)
                    nc.vector.tensor_copy(out=encT_sb, in_=encT_ps)
                    # ZT[k, t] = sum_h W[h, k] encT[h, t]
                    zt_ps = psB.tile([P, TCH], F32)
                    nc.tensor.matmul(
                        out=zt_ps,
                        lhsT=w_sb[:, :],
                        rhs=encT_sb[:, :],
                        start=True,
                        stop=True,
                    )
                    # T = tanh(ZT + Uh^T[:, ex])
                    t_sb = sbB.tile([P, TCH], HALF if BF16_T else F32)
                    nc.scalar.activation(
                        out=t_sb,
                        in_=zt_ps,
                        func=AF.Tanh,
                        bias=uh_sb[:, ex : ex + 1],
                        scale=1.0,
                    )
                    # scores[t] = sum_k T[k, t] V[k]  (t on partitions)
                    s8 = aux[:, 268 : 268 + SGROUP * CH]
                    for i in range(CH):
                        col = CH * (c % SGROUP) + i
                        nc.tensor.matmul(
                            out=s8[:, col : col + 1],
                            lhsT=t_sb[:, P * i : P * (i + 1)],
                            rhs=v_sb,
                            start=True,
                            stop=True,
                        )
                    if c % SGROUP == SGROUP - 1:
                        g2 = c // SGROUP
                        jbase = SGROUP * CH * (g2 % (NCHUNK_EX // SGROUP))
                        nc.scalar.activation(
                            out=p_sb[:, ex, jbase : jbase + SGROUP * CH],
                            in_=s8,
                            func=AF.Exp,
                        )
                        if sc == NCHUNK_EX - 1:
                            emit_softmax_tail(ex)


            # ---- flush: remaining context accumulation + c output -----------
                for cc in range(NCHUNK - CACC_DELAY, NCHUNK):
                    emit_caccum(cc)
                for ex in range(2, BL):
                    emit_c_tail(ex)
                # c_final = c_unnorm / den, then transpose to [ex, h] rows
                nc.vector.tensor_mul(
                    out=cF_sb[:, :], in0=cF_sb[:, :], in1=rden_sb[:, :BL]
                )
                nc.tensor.transpose(
                    out=aux[0:BL, 140 : 140 + H],
                    in_=cF_sb[:, :],
                    identity=ident,
                )
                nc.vector.tensor_copy(
                    out=cfin_sb[0:BL, :], in_=aux[0:BL, 140 : 140 + H]
                )
                nc.scalar.dma_start(out=c_out[:, :], in_=cfin_sb[0:BL, :])

    nc.compile()
    return nc


_CACHE: dict = {}


def _shard_inputs(encoder_out_seq, decoder_out_seq, W_a, U_a, V_a):
    enc = np.ascontiguousarray(np.asarray(encoder_out_seq, np.float32))
    dec = np.ascontiguousarray(
        np.asarray(decoder_out_seq, np.float32).reshape(B, H)
    )
    w = np.ascontiguousarray(np.asarray(W_a, np.float32))
    u = np.ascontiguousarray(np.asarray(U_a, np.float32))
    v = np.ascontiguousarray(np.asarray(V_a, np.float32))
    uh_all = dec @ u  # [B, H], tiny — computed host-side
    return [
        {
            "enc": np.ascontiguousarray(enc[BL * r : BL * (r + 1)]),
            "w_a": w,
            "v_a": v,
            "uht": np.ascontiguousarray(uh_all[BL * r : BL * (r + 1)].T),
        }
        for r in range(NCORES)
    ]


def run(inputs: dict, trace: bool = False):
    """Run the SPMD kernel; returns ((e_i, c_i), BassKernelResults)."""
    if "nc" not in _CACHE:
        _CACHE["nc"] = build_bass()
    nc = _CACHE["nc"]
    in_maps = _shard_inputs(**inputs)
    res = run_bass_kernel_spmd(nc, in_maps, list(range(NCORES)), trace=trace)
    e = np.concatenate([res.results[r]["e_out"] for r in range(NCORES)], axis=0)
    c = np.concatenate([res.results[r]["c_out"] for r in range(NCORES)], axis=0)
    return (e, c), res


def kernel(encoder_out_seq, decoder_out_seq, W_a, U_a, V_a):
    (e, c), _ = run(
        dict(
            encoder_out_seq=encoder_out_seq,
            decoder_out_seq=decoder_out_seq,
            W_a=W_a,
            U_a=U_a,
            V_a=V_a,
        )
    )
    return e, c


if __name__ == "__main__":
    nc = build_bass()
    print("built ok")
